# revision 1
# baseline (speedup 1.0000x reference)
"""GAT layer (nn_GATLayer) on 8 Trainium2 NeuronCores.

Strategy: row-shard the query/node dimension N=4096 across 8 cores
(512 rows each, flash-attention style).

Key reformulation: with u=e^{ss}, v=e^{sd}, q=e^{0.2 sd}, the masked
GAT kernel  E = m * max(u_i v_j, p_i q_j)  factors as
  E = u_i * F,   F[j,i] = m * max(v_j, q_j * w_i),  w_i = e^{-0.8 ss_i}.
The u_i factor cancels in the softmax ratio, so only F is ever
materialized, with all exps on [N]-sized vectors:
  D3 = (w_b * q_j) max v_j      one DVE tensor_scalar (4x mode) per
                                (head, j-tile); q_j/v_j live in the two
                                per-partition scalar slots
  F  = D3 * maskT               one DVE tensor_tensor (2x mode) per
                                4 heads x 2 j-tiles via broadcast
  psum65[65, 512] += h_aug^T @ F   one matmul per (head, j-tile);
                                h_aug's ones column gives the softmax
                                denominator
Mask path: ACT converts raw fp32 topology to a bf16 {1,0} natural mask
(relu(1e-9*T + 1)), PE transposes it, ACT evacuates to resident maskT.
w_all_b broadcast: K=16 selector matmuls replicate exp'd score rows to
all partitions. h/w/score production is interleaved into the attention
loops so ACT work is spread over time. No collectives: the host
concatenates the 8 disjoint row blocks.
"""
import numpy as np

N, F_IN, F_OUT, H, NCORES = 4096, 128, 64, 8, 8
NB = N // NCORES          # 512 rows per core
P = 128
NT_J = N // P             # 32 key tiles
NT_I = NB // P            # 4 own-row tiles
TI = NB                   # free-dim width of F tiles (the whole i-block)
NEG = 0.2                 # leaky relu slope
GH = 4                    # heads per group (psum banks)

_CACHE = {}


def _build_module(reps=1):
    import concourse.bacc as bacc
    import concourse.tile as tile
    import concourse.mybir as mybir
    from concourse.masks import make_identity

    dt = mybir.dt
    Alu = mybir.AluOpType
    Act = mybir.ActivationFunctionType

    nc = bacc.Bacc("TRN2", target_bir_lowering=False, debug=False,
                   num_devices=NCORES)

    x_ap = nc.dram_tensor("x", [N, F_IN], dt.float32, kind="ExternalInput").ap()
    xown_ap = nc.dram_tensor("x_own", [NB, F_IN], dt.float32,
                             kind="ExternalInput").ap()
    topo_ap = nc.dram_tensor("topo", [NB, N], dt.float32,
                             kind="ExternalInput").ap()
    proj_ap = nc.dram_tensor("proj", [H, F_IN, F_OUT], dt.float32,
                             kind="ExternalInput").ap()
    ssrc_ap = nc.dram_tensor("score_src", [H, F_OUT], dt.float32,
                             kind="ExternalInput").ap()
    sdst_ap = nc.dram_tensor("score_dst", [H, F_OUT], dt.float32,
                             kind="ExternalInput").ap()
    skw_ap = nc.dram_tensor("skip_w", [H * F_OUT, F_IN], dt.float32,
                            kind="ExternalInput").ap()
    out_ap = nc.dram_tensor("out", [NB, F_OUT], dt.float32,
                            kind="ExternalOutput").ap()

    with tile.TileContext(nc) as tc:
      for _rep in range(reps):
            with (
                tc.tile_pool(name=f"const{_rep}", bufs=1) as cpool,
                tc.tile_pool(name=f"resident{_rep}", bufs=1) as rpool,
                tc.tile_pool(name=f"stage{_rep}", bufs=2) as spool,
                tc.tile_pool(name=f"ew{_rep}", bufs=3) as epool,
                tc.tile_pool(name=f"psum{_rep}", bufs=2, space="PSUM") as ppool,
            ):
                # ---------------- constants -----------------
                id_sb = cpool.tile([P, P], dt.float32)
                make_identity(nc, id_sb[:])

                # ---------------- input staging -----------------
                xown_sb = spool.tile([P, NT_I * P], dt.float32, tag="xown",
                                     bufs=1)
                nc.sync.dma_start(
                    xown_sb.rearrange("p (k f) -> p k f", k=NT_I),
                    xown_ap.rearrange("(k p) f -> p k f", p=P))

                proj_sb = cpool.tile([P, H * F_OUT], dt.float32)
                nc.sync.dma_start(proj_sb.rearrange("p (h o) -> p h o", h=H),
                                  proj_ap.rearrange("h f o -> f h o"))
                proj_bf = cpool.tile([P, H * F_OUT], dt.bfloat16)
                nc.vector.tensor_copy(proj_bf[:], proj_sb[:])

                # score vectors transposed: rows o (x2 stacked), col h
                ssrcT = cpool.tile([P, H], dt.float32)
                sdstT = cpool.tile([P, H], dt.float32)
                nc.sync.dma_start(ssrcT[0:64, :],
                                  ssrc_ap.rearrange("h o -> o h"))
                nc.sync.dma_start(ssrcT[64:128, :],
                                  ssrc_ap.rearrange("h o -> o h"))
                nc.sync.dma_start(sdstT[0:64, :],
                                  sdst_ap.rearrange("h o -> o h"))
                nc.sync.dma_start(sdstT[64:128, :],
                                  sdst_ap.rearrange("h o -> o h"))

                # x: a small first chunk (j-tiles 0-3, so the first w-step
                # starts early) then 2 big halves [p, (k f)]
                xq0 = spool.tile([P, 4 * P], dt.float32, tag="xq0", bufs=1)
                nc.sync.dma_start(
                    xq0.rearrange("p (k f) -> p k f", k=4),
                    x_ap[0:512, :].rearrange("(k p) f -> p k f", p=P))
                x_half = []
                for g in range(2):
                    xc = spool.tile([P, 16 * P], dt.float32, tag="xchunk",
                                    bufs=2, name=f"x_half{g}")
                    nc.sync.dma_start(
                        xc.rearrange("p (k f) -> p k f", k=16),
                        x_ap[2048 * g:2048 * (g + 1), :].rearrange(
                            "(k p) f -> p k f", p=P))
                    x_half.append(xc)

                def xt_src(nt):
                    if nt < 4:
                        return xq0[:, nt * P:(nt + 1) * P]
                    return x_half[nt // 16][:, (nt % 16) * P:(nt % 16 + 1) * P]

                # skip weights: one DMA [p, (t f)]
                sw4 = spool.tile([P, 4 * F_IN], dt.float32, tag="sw", bufs=1)
                nc.sync.dma_start(sw4.rearrange("p (t f) -> p t f", t=4),
                                  skw_ap.rearrange("(t p) f -> p t f", p=P))

                # topology: one DMA per (j-group, it-pair): [p, (it2 f)]
                topo_slabs = []

                def stage_topo(jg):
                    slabs = []
                    for h2 in range(2):
                        tsl = spool.tile([P, 2 * 1024], dt.float32,
                                         tag="topo", bufs=2,
                                         name=f"topo{jg}_{h2}")
                        nc.sync.dma_start(
                            tsl.rearrange("p (it2 f) -> p it2 f", it2=2),
                            topo_ap[h2 * 256:(h2 + 1) * 256,
                                    jg * 1024:(jg + 1) * 1024].rearrange(
                                "(it2 p) f -> p it2 f", p=P))
                        slabs.append(tsl)
                    topo_slabs.append(slabs)

                stage_topo(0)

                # ---------------- xTo (transpose of own x rows) -------------
                id_bf = cpool.tile([P, P], dt.bfloat16)
                make_identity(nc, id_bf[:])
                xTo = rpool.tile([P, NB], dt.float32)     # own rows [f, i]
                tpo = ppool.tile([P, 4 * P], dt.float32, tag="big", name="xto")
                for k in range(NT_I):
                    nc.tensor.transpose(tpo[:, k * P:(k + 1) * P],
                                        xown_sb[:, k * P:(k + 1) * P],
                                        id_sb[:])
                nc.vector.tensor_copy(xTo[:], tpo[:])

                # ---------------- w vectors (proj_h @ score_h) --------------
                # w_all col q*4+c: c in {src_2q, src_2q+1, dst_2q, dst_2q+1}
                w_all = cpool.tile([P, 16], dt.float32)
                for q in range(4):
                    tp = ppool.tile([P, P], dt.float32, tag="aux", bufs=2,
                                    name=f"pjt{q}")
                    nc.tensor.transpose(tp[:], proj_sb[:, q * P:(q + 1) * P],
                                        id_sb[:])
                    pjT = spool.tile([P, P], dt.float32, tag="pjT",
                                     name=f"pjTs{q}")
                    nc.vector.tensor_copy(pjT[:], tp[:])
                    wps = ppool.tile([P, 4], dt.float32, tag="aux", bufs=2,
                                     name=f"wps{q}")
                    for c in range(4):
                        l = c % 2
                        hd = 2 * q + l
                        sc = ssrcT if c < 2 else sdstT
                        nc.tensor.matmul(wps[:, c:c + 1],
                                         pjT[l * 64:(l + 1) * 64, :],
                                         sc[l * 64:(l + 1) * 64, hd:hd + 1],
                                         start=True, stop=True)
                    nc.vector.tensor_copy(w_all[:, q * 4:(q + 1) * 4], wps[:])

                # ---- own-row s -> w_all_b (critical path, done first)
                so_sb = spool.tile([P, NT_I * 16], dt.float32, tag="sos",
                                   bufs=1)
                sps = ppool.tile([P, NT_I * 16], dt.float32, tag="aux", bufs=2,
                                 name="sop")
                for k in range(NT_I):
                    nc.tensor.matmul(sps[:, k * 16:(k + 1) * 16],
                                     xTo[:, k * P:(k + 1) * P],
                                     w_all[:], start=True, stop=True)
                nc.vector.tensor_copy(so_sb[:], sps[:])

                # query-side broadcast w_all_b = e^{-0.8 ss} per head:
                # one ACT exp on the small [128, 64] score tile, PE
                # transposes -> wT [16, 512], then per head a K=16
                # selector matmul broadcasts row src_row to all partitions.
                w_so = spool.tile([P, NT_I * 16], dt.bfloat16, tag="wso",
                                  bufs=1)
                nc.scalar.activation(w_so[:], so_sb[:], Act.Exp,
                                     bias=0.0, scale=-0.8)
                wtp = ppool.tile([16, NT_I * P], dt.bfloat16, tag="aux",
                                 bufs=2, name="wtp")
                for k in range(NT_I):
                    nc.tensor.transpose(wtp[:, k * P:(k + 1) * P],
                                        w_so[:, k * 16:(k + 1) * 16],
                                        id_bf[:])
                wT = spool.tile([16, NT_I * P], dt.bfloat16, tag="wT",
                                bufs=1)
                nc.vector.tensor_copy(wT[:], wtp[:])
                sel_all = cpool.tile([16, H * P], dt.bfloat16)
                w_all_b = rpool.tile([P, H * TI], dt.bfloat16)
                for hd in range(H):
                    src_row = 4 * (hd // 2) + hd % 2
                    # selector block = identity column src_row broadcast
                    nc.vector.tensor_copy(
                        sel_all[:, hd * P:(hd + 1) * P],
                        id_bf[0:16, src_row:src_row + 1].to_broadcast(
                            [16, P]))
                    bps = ppool.tile([P, TI], dt.float32, tag="big",
                                     name=f"bps{hd}")
                    nc.tensor.matmul(bps[:], sel_all[:, hd * P:(hd + 1) * P],
                                     wT[:], start=True, stop=True)
                    nc.vector.tensor_copy(w_all_b[:, hd * TI:(hd + 1) * TI],
                                          bps[:])

                # ------- per-chunk-group produce steps, interleaved into the
                # attention loops so ACT/PE work is spread over time.
                h_aug = rpool.tile([P, NT_J * (H * 65)], dt.bfloat16)
                h_aug4 = h_aug.rearrange("p (nt h o) -> p nt h o", h=H, o=65)
                nc.vector.memset(h_aug4[:, :, :, 64:65], 1.0)
                # s_sb[:, nt*16 + c] for all 4096 nodes
                s_sb = cpool.tile([P, NT_J * 16], dt.float32)
                sdsel = s_sb.rearrange("p (nt q c) -> p nt q c", q=4, c=4)[
                    :, :, :, 2:4]
                v_sb = cpool.tile([P, NT_J * 8], dt.float32)   # e^{sd}
                q_sb = cpool.tile([P, NT_J * 8], dt.float32)   # e^{0.2 sd}
                v4 = v_sb.rearrange("p (nt q c) -> p nt q c", q=4, c=2)
                q4 = q_sb.rearrange("p (nt q c) -> p nt q c", q=4, c=2)

                xts_bf = [None] * 8
                w_all_bf = cpool.tile([P, 16], dt.bfloat16)
                nc.vector.tensor_copy(w_all_bf[:], w_all[:])

                def produce_w_step(g2):
                    # x transposes + per-node scores + v/q exps for 4 j-tiles
                    tp = ppool.tile([P, 4 * P], dt.float32, tag="big",
                                    name=f"xt{g2}")
                    for k4 in range(4):
                        nt = g2 * 4 + k4
                        nc.tensor.transpose(tp[:, k4 * P:(k4 + 1) * P],
                                            xt_src(nt), id_sb[:])
                    xtb = spool.tile([P, 4 * P], dt.bfloat16, tag="xtb",
                                     bufs=8, name=f"xtb{g2}")
                    nc.scalar.copy(xtb[:], tp[:])
                    xts_bf[g2] = xtb
                    sps = ppool.tile([P, 4 * 16], dt.float32, tag="aux", bufs=2,
                                     name=f"sps{g2}")
                    for k4 in range(4):
                        nc.tensor.matmul(sps[:, k4 * 16:(k4 + 1) * 16],
                                         xtb[:, k4 * P:(k4 + 1) * P],
                                         w_all_bf[:], start=True, stop=True)
                    nc.scalar.copy(s_sb[:, g2 * 64:(g2 + 1) * 64], sps[:])
                    nts = slice(g2 * 4, (g2 + 1) * 4)
                    nc.scalar.activation(v4[:, nts], sdsel[:, nts], Act.Exp)
                    nc.scalar.activation(q4[:, nts], sdsel[:, nts], Act.Exp,
                                         bias=0.0, scale=0.2)

                def produce_h_step(g2, half):
                    # h for 4 j-tiles and 4 heads (half of the head dim)
                    for k4 in range(4):
                        nt = g2 * 4 + k4
                        hps = ppool.tile([P, 4 * F_OUT], dt.float32,
                                         tag="aux", bufs=2,
                                         name=f"hps{nt}_{half}")
                        nc.tensor.matmul(
                            hps[:], xts_bf[g2][:, k4 * P:(k4 + 1) * P],
                            proj_bf[:, half * 256:(half + 1) * 256],
                            start=True, stop=True)
                        nc.scalar.copy(
                            h_aug4[:, nt, 4 * half:4 * half + 4, 0:64],
                            hps.rearrange("p (h o) -> p h o", h=4))

                # ---------------- skip path: WsumT = (1/H) sum_h skip_w_h ----
                # id2[p, o] = 1 if p % 64 == o
                id2 = cpool.tile([P, 64], dt.float32)
                nc.sync.dma_start(id2[0:64, :], id_sb[0:64, 0:64])
                nc.sync.dma_start(id2[64:128, :], id_sb[0:64, 0:64])
                wsum_ps = ppool.tile([P, 64], dt.float32, tag="aux", bufs=2)
                for t in range(4):
                    nc.tensor.matmul(wsum_ps[:], sw4[:, t * F_IN:(t + 1) * F_IN],
                                     id2[:], start=(t == 0), stop=(t == 3))
                wsumT = cpool.tile([P, 64], dt.float32)
                nc.scalar.mul(wsumT[:], wsum_ps[:], 1.0 / H)

                # ---------------- resident transposed adjacency mask ---------
                maskT = rpool.tile([P, NT_J * TI], dt.bfloat16)

                # remaining topology DMAs (the DMA device drains in order).
                # {1,0}-mask conversion: j-groups 0-1 on DVE (idle during
                # startup, is_ge at 2x); groups 2-3 lazily on ACT
                # (relu(1e-9*T + 1)) so its in-order queue isn't blocked.
                for jg in range(1, 4):
                    stage_topo(jg)
                mask_nat = {}
                for jg in range(2):
                    for h2 in range(2):
                        mn = spool.tile([P, 2 * 1024], dt.bfloat16,
                                        tag="mnatv", bufs=4,
                                        name=f"mnatv{jg}_{h2}")
                        nc.vector.tensor_scalar(mn[:], topo_slabs[jg][h2][:],
                                                -0.5, None, Alu.is_ge)
                        mask_nat[(jg, h2)] = mn

                def get_mask_nat(jg, h2):
                    if (jg, h2) not in mask_nat:
                        mn = spool.tile([P, 2 * 1024], dt.bfloat16,
                                        tag="mnat", bufs=2,
                                        name=f"mnat{jg}_{h2}")
                        nc.scalar.activation(mn[:], topo_slabs[jg][h2][:],
                                             Act.Relu, bias=1.0, scale=1e-9)
                        mask_nat[(jg, h2)] = mn
                    return mask_nat[(jg, h2)]

                def build_mask_pair(jc0):
                    # both j-tiles of a pair into one [P, 1024] bf16 psum bank
                    mtp = ppool.tile([P, 2 * NT_I * P], dt.bfloat16,
                                     tag="big", name=f"mtp{jc0}")
                    for jc2 in range(2):
                        jg, jj = (jc0 + jc2) // 8, (jc0 + jc2) % 8
                        for it in range(NT_I):
                            nc.tensor.transpose(
                                mtp[:, (jc2 * NT_I + it) * P:
                                    (jc2 * NT_I + it + 1) * P],
                                get_mask_nat(jg, it // 2)[
                                    :, (it % 2) * 1024 + jj * P:
                                    (it % 2) * 1024 + (jj + 1) * P],
                                id_bf[:])
                    nc.scalar.copy(maskT[:, jc0 * TI:(jc0 + 2) * TI], mtp[:])

                # ---------------- attention: two groups of 4 heads ----------
                # per-head normalized outputs [i, (it,o)], summed in a tree
                part = [cpool.tile([P, NT_I * F_OUT], dt.float32,
                                   name=f"part{g}") for g in range(2)]
                for grp in range(2):
                    hds = [grp * GH + z for z in range(GH)]
                    ps65 = [ppool.tile([65, TI], dt.float32, tag="acc65",
                                       bufs=GH, name=f"ps65_{grp}_{z}")
                            for z in range(GH)]
                    if grp == 0:
                        for g2 in range(3):
                            produce_w_step(g2)
                        produce_h_step(0, 0)
                        produce_h_step(1, 0)
                        build_mask_pair(0)
                        build_mask_pair(2)
                    for jc0 in range(0, NT_J, 2):
                        g2c = jc0 // 4
                        if grp == 0:
                            if jc0 + 4 < NT_J:
                                build_mask_pair(jc0 + 4)
                            if jc0 % 4 == 0 and g2c + 3 < 8:
                                produce_w_step(g2c + 3)
                        if jc0 % 4 == 2 and g2c + 2 < 8:
                            produce_h_step(g2c + 2, grp)
                        # DVE: D3 = (w_b * q_j) max v_j per (head, jc);
                        # one TT applies the mask for 4 heads x 2 j-tiles
                        d3 = epool.tile([P, GH * 2 * TI], dt.bfloat16,
                                        tag="d3", bufs=2,
                                        name=f"d3_{grp}_{jc0}")
                        d34 = d3.rearrange("p (z ji) -> p z ji", z=GH)
                        for z, hd in enumerate(hds):
                            for jc2 in range(2):
                                col = (jc0 + jc2) * 8 + (hd // 2) * 2 + hd % 2
                                nc.vector.tensor_scalar(
                                    d34[:, z, jc2 * TI:(jc2 + 1) * TI],
                                    w_all_b[:, hd * TI:(hd + 1) * TI],
                                    q_sb[:, col:col + 1],
                                    v_sb[:, col:col + 1],
                                    Alu.mult, Alu.max)
                        fg = epool.tile([P, GH * 2 * TI], dt.bfloat16,
                                        tag="fg", bufs=2,
                                        name=f"fg_{grp}_{jc0}")
                        fg4 = fg.rearrange("p (z ji) -> p z ji", z=GH)
                        mb = maskT[:, jc0 * TI:(jc0 + 2) * TI].unsqueeze(
                            1).to_broadcast([P, GH, 2 * TI])
                        nc.vector.tensor_mul(fg4, d34, mb)
                        for z, hd in enumerate(hds):
                            for jc2 in range(2):
                                jc = jc0 + jc2
                                nc.tensor.matmul(
                                    ps65[z][:], h_aug4[:, jc, hd, :],
                                    fg4[:, z, jc2 * TI:(jc2 + 1) * TI],
                                    start=(jc == 0), stop=(jc == NT_J - 1))

                    if grp == 0:
                        # group 1's first h blocks, ahead of its loop so the
                        # normalize below doesn't delay them on ACT's queue
                        produce_h_step(0, 1)
                        produce_h_step(1, 1)

                    # normalize this group: transpose, recip, ACT-scaled evac
                    nrm = []
                    for z, hd in enumerate(hds):
                        nd_sb = spool.tile([65, TI], dt.float32, tag="nd",
                                           bufs=2, name=f"nd{hd}")
                        nc.scalar.copy(nd_sb[:], ps65[z][:])
                        tpn = ppool.tile([P, NT_I * 65], dt.float32,
                                         tag="aux", bufs=2, name=f"tpn{hd}")
                        for it in range(NT_I):
                            nc.tensor.transpose(
                                tpn[:, it * 65:(it + 1) * 65],
                                nd_sb[:, it * P:(it + 1) * P],
                                id_sb[0:65, 0:65])
                        rc = spool.tile([P, NT_I], dt.float32, tag="rc",
                                        bufs=2, name=f"rc{hd}")
                        tpn4 = tpn.rearrange("p (it o) -> p it o", it=NT_I)
                        nc.vector.reciprocal(rc[:], tpn4[:, :, 64])
                        nr = spool.tile([P, NT_I * F_OUT], dt.float32,
                                        tag="nrm", bufs=GH, name=f"nrm{hd}")
                        for it in range(NT_I):
                            nc.scalar.activation(
                                nr[:, it * F_OUT:(it + 1) * F_OUT],
                                tpn4[:, it, 0:64], Act.Identity,
                                bias=0.0, scale=rc[:, it:it + 1])
                        nrm.append(nr)
                    # head-sum tree for this group
                    t01 = spool.tile([P, NT_I * F_OUT], dt.float32,
                                     tag="t01", bufs=1, name=f"t01_{grp}")
                    t23 = spool.tile([P, NT_I * F_OUT], dt.float32,
                                     tag="t23", bufs=1, name=f"t23_{grp}")
                    nc.vector.tensor_add(t01[:], nrm[0][:], nrm[1][:])
                    nc.vector.tensor_add(t23[:], nrm[2][:], nrm[3][:])
                    nc.vector.tensor_add(part[grp][:], t01[:], t23[:])

                # ---------------- skip + mean + leaky relu + out -------------
                tot = cpool.tile([P, NT_I * F_OUT], dt.float32)
                nc.vector.tensor_add(tot[:], part[0][:], part[1][:])
                tot4 = tot.rearrange("p (it o) -> p it o", it=NT_I)
                for it in range(NT_I):
                    skp = ppool.tile([P, 64], dt.float32, tag="aux", bufs=2,
                                     name=f"skp{it}")
                    nc.tensor.matmul(skp[:], xTo[:, it * P:(it + 1) * P],
                                     wsumT[:], start=True, stop=True)
                    qs = spool.tile([P, F_OUT], dt.float32, tag="qs",
                                    bufs=2, name=f"qs{it}")
                    # qs = tot/H + skip
                    nc.vector.scalar_tensor_tensor(
                        qs[:], tot4[:, it, :], 1.0 / H, skp[:],
                        Alu.mult, Alu.add)
                    ot = spool.tile([P, F_OUT], dt.float32, tag="ot",
                                    bufs=2, name=f"ot{it}")
                    # leaky = max(qs, 0.2*qs)
                    nc.vector.scalar_tensor_tensor(
                        ot[:], qs[:], NEG, qs[:], Alu.mult, Alu.max)
                    nc.sync.dma_start(out_ap[it * P:(it + 1) * P, :], ot[:])

    nc.compile()
    return nc


def _get_module(reps=1):
    if reps not in _CACHE:
        _CACHE[reps] = _build_module(reps)
    return _CACHE[reps]


def _make_in_maps(x, topology, proj, score_src, score_dst, skip_w):
    x = np.ascontiguousarray(x, dtype=np.float32)
    topology = np.ascontiguousarray(topology, dtype=np.float32)
    proj = np.ascontiguousarray(proj, dtype=np.float32)
    score_src = np.ascontiguousarray(score_src, dtype=np.float32)
    score_dst = np.ascontiguousarray(score_dst, dtype=np.float32)
    skip_w = np.ascontiguousarray(skip_w, dtype=np.float32)
    in_maps = []
    for c in range(NCORES):
        in_maps.append({
            "x": x,
            "x_own": np.ascontiguousarray(x[c * NB:(c + 1) * NB, :]),
            "topo": np.ascontiguousarray(topology[c * NB:(c + 1) * NB, :]),
            "proj": proj,
            "score_src": score_src,
            "score_dst": score_dst,
            "skip_w": skip_w,
        })
    return in_maps


def _fast_runner(nc):
    """Cached sharded PJRT runner (same _bass_exec_p path that
    run_bass_kernel_spmd uses under axon, but jit-cached across calls)."""
    import jax
    from jax.sharding import Mesh, PartitionSpec
    from jax.experimental.shard_map import shard_map
    from concourse import mybir
    from concourse.bass2jax import (_bass_exec_p, partition_id_tensor,
                                    install_neuronx_cc_hook)

    install_neuronx_cc_hook()
    partition_name = (nc.partition_id_tensor.name
                      if nc.partition_id_tensor else None)
    in_names, out_names, out_avals = [], [], []
    for alloc in nc.m.functions[0].allocations:
        if not isinstance(alloc, mybir.MemoryLocationSet):
            continue
        name = alloc.memorylocations[0].name
        if alloc.kind == "ExternalInput":
            if name != partition_name:
                in_names.append(name)
        elif alloc.kind == "ExternalOutput":
            out_names.append(name)
            out_avals.append(jax.core.ShapedArray(
                tuple(alloc.tensor_shape), mybir.dt.np(alloc.dtype)))
    n_params = len(in_names)
    all_in = list(in_names) + list(out_names)
    if partition_name is not None:
        all_in.append(partition_name)

    def _body(*args):
        operands = list(args)
        if partition_name is not None:
            operands.append(partition_id_tensor())
        return tuple(_bass_exec_p.bind(
            *operands, out_avals=tuple(out_avals), in_names=tuple(all_in),
            out_names=tuple(out_names), lowering_input_output_aliases=(),
            sim_require_finite=True, sim_require_nnan=True, nc=nc))

    devices = jax.devices()[:NCORES]
    mesh = Mesh(np.asarray(devices), ("core",))
    specs_in = (PartitionSpec("core"),) * (n_params + len(out_names))
    specs_out = (PartitionSpec("core"),) * len(out_names)
    fn = jax.jit(shard_map(_body, mesh=mesh, in_specs=specs_in,
                           out_specs=specs_out, check_rep=False),
                 keep_unused=True)
    zero_shapes = [(NCORES * a.shape[0], *a.shape[1:]) for a in out_avals]
    zero_dtypes = [a.dtype for a in out_avals]

    def run(in_maps):
        per_core = [[np.asarray(m[nm]) for nm in in_names] for m in in_maps]
        cin = [np.concatenate([per_core[c][i] for c in range(NCORES)], axis=0)
               for i in range(n_params)]
        cz = [np.zeros(s, d) for s, d in zip(zero_shapes, zero_dtypes)]
        outs = fn(*cin, *cz)
        o = np.asarray(outs[out_names.index("out")])
        return o.reshape(NCORES, NB, F_OUT)

    return run


def kernel(x, topology, proj, score_src, score_dst, skip_w):
    nc = _get_module()
    in_maps = _make_in_maps(x, topology, proj, score_src, score_dst, skip_w)
    if "runner" not in _CACHE:
        # first call: go through the canonical entry point
        from concourse.bass_utils import run_bass_kernel_spmd
        res = run_bass_kernel_spmd(nc, in_maps, list(range(NCORES)))
        _CACHE["runner"] = _fast_runner(nc)
        out = np.concatenate([res.results[c]["out"] for c in range(NCORES)],
                             axis=0)
        return out.astype(np.float32)
    per_core = _CACHE["runner"](in_maps)
    return np.concatenate(list(per_core), axis=0).astype(np.float32)

